# revision 12
# baseline (speedup 1.0000x reference)
"""DCNv2 (modulated deformable conv 3x3) for Trainium2, 8 NeuronCores.

Sharding: pure data-parallel over batch B=8 -> core b computes batch b.

Per-core algorithm (batch b, C=Cout=128, H=W=64, P=H*W=4096):
  1. PE (bf16): offset/mask conv as 9 accumulated matmuls over a zero-padded
     channel-major x ([128, 66*66] SBUF), output [41, P] channel-major
     (channels: 0:9 y-offsets, 9:18 x-offsets, 32:41 mask - 32-aligned for
     the engines' base-partition restriction).  ACT applies bias (+ sigmoid
     for mask rows) during PSUM evacuation.
  2. PE transposes [41,128] chunks -> p-major planes [128(p), 41, 32(pb)].
  3. DVE: bilinear coefficient planes.  floor() via the fp32 round trick
     (x - 0.5 + 1.5*2^23) - 1.5*2^23.  Per kernel-point k one gather index
     idx = ysel*64 + xsel and four per-corner coefficients
     C[yl][xl] = mask * ylane_yl * xlane_xl (border clip/zero semantics
     folded into the lane coefficients).
  4. Index wrap for the gather engine ([16-partition wrap, replicated to 8
     Q7 core groups]) built with 11 bulk DMAs covering all (k, half).
  5. GPSIMD dma_gather (transpose=False) over a host-packed bf16 table
     x2[p] = [x[p], x[p+1], x[p+64], x[p+65]] ([P, 512] in DRAM): each
     int16 index fetches 1KB = four bilinear corners x 128 channels,
     landing p-major: gt[p, j, 4*128] for position p = h*2048 + j*128 + part.
     One 2048-index gather per (k, half), round-robin over 4 SWDGE queues.
  6. Corner combine in [p, c] layout with per-partition scalars (bf16):
     ACT: a0 = g00*C00, a1 = g10*C10 (scaled copies);
     DVE: u0 = g01*C01 + a0, u1 = g11*C11 + a1 (STT); V = u0 + u1 (TT).
  7. PE: transpose V[p,c] -> VT[c,p] (bf16, PSUM), ACT evacuates 4 at a
     time to SBUF; matmul pa[o, p] += W_k[c, o].T @ VT[c, p] accumulated
     over k=0..8 directly in PSUM (start/stop).
  8. ACT evacuates pa -> [128(o), P] SBUF; DMA out channel-major
     ([Cout, P] in DRAM; host reshapes to [Cout, H, W] with no transpose).
"""

import sys

sys.path.insert(0, "/opt/trn_rl_repo")

from contextlib import ExitStack

import numpy as np
import ml_dtypes

import concourse.bacc as bacc
import concourse.bass as bass
import concourse.mybir as mybir
import concourse.tile as tile
from concourse.ap import AP
from concourse.bass import ts
from concourse.bass_utils import run_bass_kernel_spmd
from concourse.library_config import mlp as mlp_lib
from concourse.masks import make_identity

F32 = mybir.dt.float32
BF16 = mybir.dt.bfloat16
I16 = mybir.dt.int16

B, C, H, W = 8, 128, 64, 64
COUT = 128
K2 = 9
P = H * W            # 4096
NPB = P // 128       # 32 p-blocks
NH = 2               # halves of P (one 2048-idx gather per (k, half))
NJ = 16              # p-blocks per half
HP = H + 2           # padded side
MAGIC = 12582912.0   # 1.5 * 2**23
AOP = mybir.AluOpType
AF = mybir.ActivationFunctionType

_CACHE = {}


def _build():
    nc = bacc.Bacc("TRN2", target_bir_lowering=False, num_swdge_queues=4)

    xpad_d = nc.dram_tensor("xpad", [128, HP * HP], BF16, kind="ExternalInput")
    x2_d = nc.dram_tensor("x2rows", [P, 512], BF16, kind="ExternalInput")
    wmain_d = nc.dram_tensor("wmain", [128, K2, COUT], BF16, kind="ExternalInput")
    woff_d = nc.dram_tensor("woff", [128, K2, 41], BF16, kind="ExternalInput")
    bias_d = nc.dram_tensor("bias41", [41, 1], F32, kind="ExternalInput")
    byk_d = nc.dram_tensor("byk", [128, K2, NPB], F32, kind="ExternalInput")
    bxk_d = nc.dram_tensor("bxk", [128, K2, NPB], F32, kind="ExternalInput")
    out_d = nc.dram_tensor("out", [COUT, P], F32, kind="ExternalOutput")

    with tile.TileContext(nc) as tc:
        with (
            tc.tile_pool(name="const", bufs=1) as cp,
            tc.tile_pool(name="coef", bufs=1) as cf,
            tc.tile_pool(name="gp", bufs=4) as gp,
            tc.tile_pool(name="vp", bufs=3) as vp,
            tc.tile_pool(name="vts", bufs=3) as vs,
        ):
            nc.gpsimd.load_library(mlp_lib)

            # ---- constant loads (SP-engine HWDGE queues, off gpsimd) ----
            xpad = cp.tile([128, HP * HP], BF16)
            wo = cp.tile([128, K2, 41], BF16)
            nc.sync.dma_start(wo[:], woff_d[:])
            nc.sync.dma_start(xpad[:], xpad_d[:])
            wm = cp.tile([128, K2, COUT], BF16)
            nc.sync.dma_start(wm[:], wmain_d[:])
            bias = cp.tile([41, 1], F32)
            nc.sync.dma_start(bias[:], bias_d[:])
            byk = cp.tile([128, K2, NPB], F32)
            nc.sync.dma_start(byk[:], byk_d[:])
            bxk = cp.tile([128, K2, NPB], F32)
            nc.sync.dma_start(bxk[:], bxk_d[:])
            ident41 = cp.tile([64, 64], F32)
            make_identity(nc, ident41[:])
            identB = cp.tile([128, 128], BF16)
            make_identity(nc, identB[:])

            # ---- offset/mask conv: [41, P] channel-major (bf16 matmuls) ----
            _es1 = ExitStack()
            psO = _es1.enter_context(tc.tile_pool(name="psO", bufs=2, space="PSUM"))
            psT = _es1.enter_context(tc.tile_pool(name="psT", bufs=2, space="PSUM"))
            offs_cm = cf.tile([41, P], F32)
            nc.gpsimd.memset(offs_cm[:], 0.0)
            xv = xpad[:].rearrange("c (h w) -> c h w", h=HP)
            for ch in range(8):
                po = psO.tile([41, 512], F32)
                r0 = ch * 8
                for k in range(K2):
                    ki, kj = k // 3, k % 3
                    rhs = xv[:, r0 + ki : r0 + ki + 8, kj : kj + W]
                    nc.tensor.matmul(
                        po[:], wo[:, k, :], rhs,
                        start=(k == 0), stop=(k == K2 - 1),
                    )
                sl = slice(ch * 512, (ch + 1) * 512)
                nc.scalar.activation(
                    offs_cm[0:18, sl], po[0:18, :], AF.Identity,
                    bias=bias[0:18, :], scale=1.0,
                )
                nc.scalar.activation(
                    offs_cm[32:41, sl], po[32:41, :], AF.Sigmoid,
                    bias=bias[32:41, :], scale=1.0,
                )

            # ---- transpose to p-major [128, 41, 32] ----
            offs_pm = cf.tile([128, 41, NPB], F32)
            for t in range(NPB):
                pt = psT.tile([128, 41], F32)
                nc.tensor.transpose(
                    pt[:], offs_cm[:, ts(t, 128)], ident41[:41, :41]
                )
                nc.vector.tensor_copy(offs_pm[:, :, t], pt[:])

            offy = offs_pm[:, 0:9, :]
            offx = offs_pm[:, 9:18, :]
            mask = offs_pm[:, 32:41, :]

            # ---- coefficient planes (DVE, [128, 9, 32] each) ----
            SH = [128, K2, NPB]
            _tln = [0]

            def tl():
                _tln[0] += 1
                return cf.tile(SH, F32, name=f"cftmp{_tln[0]}")

            def TS(out, in0, s1, op0, s2=None, op1=None):
                kw = {"op1": op1} if op1 is not None else {}
                nc.vector.tensor_scalar(
                    out=out, in0=in0, scalar1=s1, scalar2=s2, op0=op0, **kw
                )

            def TT(out, a, b, op):
                nc.vector.tensor_tensor(out=out, in0=a, in1=b, op=op)

            # index chain first (gathers depend only on this)
            t0 = tl(); TS(t0[:], offy, -0.5, AOP.add, MAGIC, AOP.add)
            iy = tl(); TS(iy[:], t0[:], MAGIC, AOP.subtract)
            ys0 = tl(); TT(ys0[:], iy[:], byk[:], AOP.add)
            ysel = tl(); TS(ysel[:], ys0[:], 0.0, AOP.max, 62.0, AOP.min)
            t1 = tl(); TS(t1[:], offx, -0.5, AOP.add, MAGIC, AOP.add)
            ix = tl(); TS(ix[:], t1[:], MAGIC, AOP.subtract)
            xs0 = tl(); TT(xs0[:], ix[:], bxk[:], AOP.add)
            xst = tl(); TS(xst[:], xs0[:], 0.0, AOP.max, 62.0, AOP.min)
            ib = tl()
            nc.vector.scalar_tensor_tensor(
                out=ib[:], in0=ysel[:], scalar=64.0, in1=xst[:],
                op0=AOP.mult, op1=AOP.add,
            )
            # idx16 [128, k, h, j]: p = h*2048 + j*128 + part
            idx16 = cf.tile([128, K2, NH, NJ], I16)
            nc.vector.tensor_copy(
                idx16[:], ib[:].rearrange("p k (h j) -> p k h j", h=NH)
            )
            # wrap for the gather engine: idxw[w, k, h, j*8 + g] holds the
            # index of position p = h*2048 + j*128 + g*16 + w, then the
            # 16-partition block is replicated to all 8 Q7 core groups.
            idxw = cf.tile([128, K2, NH, 128], I16)
            for k in range(K2):
                for g in range(8):
                    nc.sync.dma_start(
                        idxw[0:16, k, :, g:128:8],
                        idx16[16 * g : 16 * (g + 1), k, :, :],
                    )
                for np2 in (16, 32, 64):
                    nc.sync.dma_start(idxw[np2 : 2 * np2, k], idxw[0:np2, k])

            # remaining coefficient math
            fy = tl(); TT(fy[:], offy, iy[:], AOP.subtract)
            ys1 = tl(); TS(ys1[:], ys0[:], 1.0, AOP.add)
            yc0 = tl(); TS(yc0[:], ys0[:], 0.0, AOP.max, 63.0, AOP.min)
            yc1 = tl(); TS(yc1[:], ys1[:], 0.0, AOP.max, 63.0, AOP.min)
            vy0 = tl(); TT(vy0[:], yc0[:], ys0[:], AOP.is_equal)
            vy1 = tl(); TT(vy1[:], yc1[:], ys1[:], AOP.is_equal)
            gy = tl(); TS(gy[:], fy[:], -1.0, AOP.mult, 1.0, AOP.add)
            wy0 = tl(); TT(wy0[:], gy[:], vy0[:], AOP.mult)
            wy1 = tl(); TT(wy1[:], fy[:], vy1[:], AOP.mult)
            f0 = tl(); TT(f0[:], ysel[:], ys0[:], AOP.is_equal)
            fm = tl(); TS(fm[:], ys0[:], -1.0, AOP.is_equal)
            fp = tl(); TS(fp[:], ys0[:], 63.0, AOP.is_equal)
            ya = tl(); TT(ya[:], wy0[:], f0[:], AOP.mult)
            yb = tl(); TT(yb[:], wy1[:], fm[:], AOP.mult)
            ylane0 = tl(); TT(ylane0[:], ya[:], yb[:], AOP.add)
            yc_ = tl(); TT(yc_[:], wy1[:], f0[:], AOP.mult)
            yd = tl(); TT(yd[:], wy0[:], fp[:], AOP.mult)
            ylane1 = tl(); TT(ylane1[:], yc_[:], yd[:], AOP.add)
            myl0 = tl(); TT(myl0[:], ylane0[:], mask, AOP.mult)
            myl1 = tl(); TT(myl1[:], ylane1[:], mask, AOP.mult)
            # x side
            fx = tl(); TT(fx[:], offx, ix[:], AOP.subtract)
            xs1 = tl(); TS(xs1[:], xs0[:], 1.0, AOP.add)
            xc0 = tl(); TS(xc0[:], xs0[:], 0.0, AOP.max, 63.0, AOP.min)
            xc1 = tl(); TS(xc1[:], xs1[:], 0.0, AOP.max, 63.0, AOP.min)
            vx0 = tl(); TT(vx0[:], xc0[:], xs0[:], AOP.is_equal)
            vx1 = tl(); TT(vx1[:], xc1[:], xs1[:], AOP.is_equal)
            gx = tl(); TS(gx[:], fx[:], -1.0, AOP.mult, 1.0, AOP.add)
            wx0 = tl(); TT(wx0[:], gx[:], vx0[:], AOP.mult)
            wx1 = tl(); TT(wx1[:], fx[:], vx1[:], AOP.mult)
            e0 = tl(); TT(e0[:], xst[:], xs0[:], AOP.is_equal)
            em = tl(); TS(em[:], xs0[:], -1.0, AOP.is_equal)
            ep = tl(); TS(ep[:], xs0[:], 63.0, AOP.is_equal)
            l0a = tl(); TT(l0a[:], wx0[:], e0[:], AOP.mult)
            l0b = tl(); TT(l0b[:], wx1[:], em[:], AOP.mult)
            xlane0 = tl(); TT(xlane0[:], l0a[:], l0b[:], AOP.add)
            l1a = tl(); TT(l1a[:], wx1[:], e0[:], AOP.mult)
            l1b = tl(); TT(l1b[:], wx0[:], ep[:], AOP.mult)
            xlane1 = tl(); TT(xlane1[:], l1a[:], l1b[:], AOP.add)
            # final per-corner coefficients (table corner order:
            # (y0,x0), (y0,x1), (y1,x0), (y1,x1))
            C00 = tl(); TT(C00[:], myl0[:], xlane0[:], AOP.mult)
            C01 = tl(); TT(C01[:], myl0[:], xlane1[:], AOP.mult)
            C10 = tl(); TT(C10[:], myl1[:], xlane0[:], AOP.mult)
            C11 = tl(); TT(C11[:], myl1[:], xlane1[:], AOP.mult)

            _es1.close()
            _es2 = ExitStack()
            psA = _es2.enter_context(tc.tile_pool(name="psA", bufs=1, space="PSUM"))
            psV = _es2.enter_context(tc.tile_pool(name="psV", bufs=2, space="PSUM"))

            # ---- main loop ----
            outs = cf.tile([128, NH, NJ, 128], F32)

            src_ap = AP(
                tensor=x2_d[:].tensor, offset=0, ap=[[512, P], [1, 512]]
            )
            for h in range(NH):
                pa = psA.tile([128, NJ, 128], F32)
                for k in range(K2):
                    gt = gp.tile([128, NJ, 512], BF16, tag="G")
                    # 512-idx chunks: >512 idxs per gather overflows the
                    # descriptor ring carveout and wedges the device.
                    qn = (h * K2 + k) % 4
                    for c in range(4):
                        nc.gpsimd.dma_gather(
                            gt[:, c * 4 : (c + 1) * 4, :], src_ap,
                            idxw[:, k, h, c * 32 : (c + 1) * 32],
                            512, 512,
                            elem_size=512, elem_step=512, transpose=False,
                            queue_num=qn,
                        )
                    gv = gt[:].rearrange("p j (cn c) -> p j cn c", cn=4)
                    for j4 in range(NJ // 4):
                        pv = psV.tile([128, 4, 128], BF16)
                        for jj in range(4):
                            j = j4 * 4 + jj
                            pb = h * NJ + j
                            a0 = vp.tile([128, 128], BF16, tag="a0")
                            nc.scalar.activation(
                                a0[:], gv[:, j, 0, :], AF.Copy,
                                scale=C00[:, k, pb : pb + 1],
                            )
                            a1 = vp.tile([128, 128], BF16, tag="a1")
                            nc.scalar.activation(
                                a1[:], gv[:, j, 2, :], AF.Copy,
                                scale=C10[:, k, pb : pb + 1],
                            )
                            u0 = vp.tile([128, 128], BF16, tag="u0")
                            nc.vector.scalar_tensor_tensor(
                                out=u0[:], in0=gv[:, j, 1, :],
                                scalar=C01[:, k, pb : pb + 1], in1=a0[:],
                                op0=AOP.mult, op1=AOP.add,
                            )
                            u1 = vp.tile([128, 128], BF16, tag="u1")
                            nc.vector.scalar_tensor_tensor(
                                out=u1[:], in0=gv[:, j, 3, :],
                                scalar=C11[:, k, pb : pb + 1], in1=a1[:],
                                op0=AOP.mult, op1=AOP.add,
                            )
                            v = vp.tile([128, 128], BF16, tag="V")
                            nc.vector.tensor_tensor(
                                out=v[:], in0=u0[:], in1=u1[:], op=AOP.add
                            )
                            nc.tensor.transpose(pv[:, jj, :], v[:], identB[:])
                        vt = vs.tile([128, 4, 128], BF16, tag="VT")
                        nc.scalar.activation(vt[:], pv[:], AF.Copy)
                        # one accumulation group per 2KB PSUM bank (4 j's):
                        # start marks the whole bank pending-zero, so only
                        # the first matmul touching the bank starts, and
                        # only the last one stops.
                        for jj in range(4):
                            j = j4 * 4 + jj
                            nc.tensor.matmul(
                                pa[:, j, :], wm[:, k, :], vt[:, jj, :],
                                start=(k == 0 and jj == 0),
                                stop=(k == K2 - 1 and jj == 3),
                            )
                nc.scalar.activation(outs[:, h, :, :], pa[:], AF.Copy)

            nc.sync.dma_start(
                out_d[:].rearrange("o (h j p) -> o h j p", h=NH, j=NJ),
                outs[:],
            )
            _es2.close()

    nc.compile()
    return nc


def _host_prep(x, weight, offset_w, offset_b, mask_w, mask_b):
    x = np.asarray(x, np.float32)
    weight = np.asarray(weight, np.float32)
    offset_w = np.asarray(offset_w, np.float32)
    offset_b = np.asarray(offset_b, np.float32)
    mask_w = np.asarray(mask_w, np.float32)
    mask_b = np.asarray(mask_b, np.float32)

    wmain = np.ascontiguousarray(
        np.transpose(weight.reshape(COUT, C, K2), (1, 2, 0))
    ).astype(ml_dtypes.bfloat16)
    ow = offset_w.reshape(18, C, K2)
    w41 = np.zeros((41, C, K2), np.float32)
    w41[0:9] = ow[0::2]
    w41[9:18] = ow[1::2]
    w41[32:41] = mask_w.reshape(9, C, K2)
    woff = np.ascontiguousarray(np.transpose(w41, (1, 2, 0))).astype(
        ml_dtypes.bfloat16
    )
    bias41 = np.zeros((41, 1), np.float32)
    bias41[0:9, 0] = offset_b[0::2]
    bias41[9:18, 0] = offset_b[1::2]
    bias41[32:41, 0] = mask_b

    ps = np.arange(P)
    ho = (ps // W).reshape(NPB, 128).T.astype(np.float32)
    wo_ = (ps % W).reshape(NPB, 128).T.astype(np.float32)
    byk = np.empty((128, K2, NPB), np.float32)
    bxk = np.empty((128, K2, NPB), np.float32)
    for k in range(K2):
        byk[:, k, :] = ho + (k // 3 - 1)
        bxk[:, k, :] = wo_ + (k % 3 - 1)

    shared = dict(wmain=wmain, woff=woff, bias41=bias41, byk=byk, bxk=bxk)

    in_maps = []
    for b in range(B):
        xpad = np.zeros((C, HP, HP), ml_dtypes.bfloat16)
        xpad[:, 1 : H + 1, 1 : W + 1] = x[b].astype(ml_dtypes.bfloat16)
        xr = np.zeros((P + 66, C), ml_dtypes.bfloat16)
        xr[:P] = x[b].transpose(1, 2, 0).reshape(P, C).astype(ml_dtypes.bfloat16)
        x2 = np.ascontiguousarray(
            np.concatenate(
                [xr[0:P], xr[1 : P + 1], xr[64 : P + 64], xr[65 : P + 65]],
                axis=1,
            )
        )
        in_maps.append(
            dict(xpad=xpad.reshape(C, HP * HP), x2rows=x2, **shared)
        )
    return in_maps


def kernel(x, weight, offset_w, offset_b, mask_w, mask_b):
    if "nc" not in _CACHE:
        _CACHE["nc"] = _build()
    nc = _CACHE["nc"]
    in_maps = _host_prep(x, weight, offset_w, offset_b, mask_w, mask_b)
    res = run_bass_kernel_spmd(nc, in_maps, list(range(B)))
    _CACHE["last_result"] = res
    out = np.empty((B, COUT, H, W), np.float32)
    for b in range(B):
        out[b] = res.results[b]["out"].reshape(COUT, H, W)
    return out
